# revision 1
# baseline (speedup 1.0000x reference)
"""BehaviorMoE Trainium2 kernel (8 NeuronCores, SPMD data-parallel over sorted tokens).

Contract: kernel(**inputs) takes FULL inputs as returned by setup_inputs() and
returns the FULL [8192, 1024] float32 output.

Strategy:
  - Host: sort tokens by behavior id. Tokens with b==0 need no expert compute
    (output = x + beta); they are used as masked filler so that every core gets
    exactly 1024 tokens that share a single behavior id.  Each core receives
    the stacked weight matrix [W_sh0; W_sh1; W_sh2; W_sp[t]]^T for its behavior.
  - Device (identical SPMD program, per-core data):
      Phase B (gates): per 128-token tile, gate logits (PE), masked softmax
        (DVE/ACT), PE transpose of gates, bias combine via gates^T @ b_all (PE)
        copied into an SBUF accumulator.
      Phase C (experts): e-outer loop streams the stacked weights once while
        the PE runs a dense fp32r matmul stream; a fused DVE
        scalar_tensor_tensor accumulates gate-weighted expert outputs into
        ping-pong SBUF accumulators (in-place DVE ops fault on this HW).
      Phase D (tail): LayerNorm stats (ACT Square batched to avoid act-table
        reloads), normalize + residual, DMA out.
  - Host: scatter per-core outputs back to original token order.
"""

import os
import sys

import numpy as np

for _p in ("/opt/trn_rl_repo", "/root/.axon_site/_ro/trn_rl_repo"):
    if os.path.isdir(_p) and _p not in sys.path:
        sys.path.append(_p)

from contextlib import ExitStack

from concourse import bacc, bass, masks, mybir, tile
from concourse.bass_utils import run_bass_kernel_spmd

F32 = mybir.dt.float32
F32R = mybir.dt.float32r
AX = mybir.AxisListType
ALU = mybir.AluOpType
ACTF = mybir.ActivationFunctionType

D = 1024            # model dim
N = 8192            # tokens
NB = 4              # behaviors
NESH = 3            # shared experts
NE = 4              # experts per behavior (3 shared + 1 specific)
EPS = 1e-5
NCORES = 8
M = N // NCORES     # tokens per core
KT = D // 128       # k tiles (contraction)
IT = M // 128       # token tiles per core
FH = 512            # feature half-tile (psum bank width in f32)


def _build_program(trivial_affine: bool) -> bass.Bass:
    nc = bacc.Bacc()

    xt_d = nc.declare_dram_parameter("xt", [KT, 128, M], F32R, isOutput=False)
    xtok_d = nc.declare_dram_parameter("xtok", [M, D], F32, isOutput=False)
    wt_d = nc.declare_dram_parameter("wt", [NE, 2, KT, 128, FH], F32R, isOutput=False)
    wg_d = nc.declare_dram_parameter("wg", [128, KT * 128], F32R, isOutput=False)
    ball_d = nc.declare_dram_parameter("ball", [128, D], F32R, isOutput=False)
    mask_d = nc.declare_dram_parameter("mask", [128, IT], F32, isOutput=False)
    if not trivial_affine:
        gam_d = nc.declare_dram_parameter("gam", [128, D], F32, isOutput=False)
        bet_d = nc.declare_dram_parameter("bet", [128, D], F32, isOutput=False)
    out_d = nc.declare_dram_parameter("out", [M, D], F32, isOutput=True)

    with tile.TileContext(nc) as tc, ExitStack() as ctx:
        const = ctx.enter_context(tc.tile_pool(name="const", bufs=1))
        xtp = ctx.enter_context(tc.tile_pool(name="xt", bufs=KT))
        wpool = ctx.enter_context(tc.tile_pool(name="w", bufs=16))
        selp = ctx.enter_context(tc.tile_pool(name="sel", bufs=2 * IT))
        xtokp = ctx.enter_context(tc.tile_pool(name="xtok", bufs=3))
        outp = ctx.enter_context(tc.tile_pool(name="outp", bufs=2))
        scrp = ctx.enter_context(tc.tile_pool(name="scr", bufs=3))
        gatep = ctx.enter_context(tc.tile_pool(name="gate", bufs=IT))
        smallp = ctx.enter_context(tc.tile_pool(name="small", bufs=40))
        zpool = ctx.enter_context(tc.tile_pool(name="z", bufs=5, space="PSUM"))
        pspool = ctx.enter_context(tc.tile_pool(name="ps", bufs=3, space="PSUM"))

        # ---- PE warm-up source ----
        zsrc0 = const.tile([128, FH], F32, tag="zsrc0")
        nc.gpsimd.memset(zsrc0[:], 0.0)
        zsrc = const.tile([128, FH], F32R, tag="zsrc")
        nc.vector.tensor_copy(zsrc[:], zsrc0[:])

        dummy_state = {"n": 0}

        def dummies(n):
            """n PE filler matmuls (zero @ zero) keeping the PE array busy."""
            dt_ = pspool.tile([128, FH], F32, tag="ps", name=f"dps{dummy_state['n']}")
            dummy_state["n"] += 1
            for j in range(n):
                nc.tensor.matmul(
                    dt_[:], zsrc[:, 0:128], zsrc[:],
                    start=(j == 0), stop=(j == n - 1),
                )

        # ---- small constant inputs ----
        wg_sb = const.tile([128, KT * 128], F32R, tag="wg")
        nc.sync.dma_start(wg_sb[:], wg_d[:])
        ball_sb = const.tile([128, D], F32R, tag="ball")
        nc.sync.dma_start(ball_sb[:], ball_d[:])
        mask_sb = const.tile([128, IT], F32, tag="mask")
        nc.sync.dma_start(mask_sb[:], mask_d[:])
        if not trivial_affine:
            gam_sb = const.tile([128, D], F32, tag="gam")
            nc.sync.dma_start(gam_sb[:], gam_d[:])
            bet_sb = const.tile([128, D], F32, tag="bet")
            nc.sync.dma_start(bet_sb[:], bet_d[:])

        # ---- resident xT k-tiles first, then streamed weight half-tiles ----
        xT = []
        for k in range(KT):
            t = xtp.tile([128, M], F32R, tag="xt")
            nc.sync.dma_start(t[:], xt_d[k])
            xT.append(t)
        w_sb = {}
        for e in range(NE):
            for c in (0, 1):
                for k in range(KT):
                    t = wpool.tile([128, FH], F32R, tag="w", name=f"w{e}{c}{k}")
                    nc.sync.dma_start(t[:], wt_d[e, c, k])
                    w_sb[(e, c, k)] = t

        identity = const.tile([128, 128], F32, tag="ident")
        masks.make_identity(nc, identity[:])
        identR = const.tile([128, 128], F32R, tag="identR")
        nc.vector.tensor_copy(identR[:], identity[:])

        # ---- accumulators (ping-pong; in-place DVE ops fault) ----
        selA = [selp.tile([128, D], F32, tag="sel", name=f"selA{i}") for i in range(IT)]
        selB = [selp.tile([128, D], F32, tag="sel", name=f"selB{i}") for i in range(IT)]

        # ---- gate logits glT[4, tok], k-paced by the xT DMAs; dummy
        #      matmuls keep the PE dense so the HAM never sees sparse work ----
        dummies(8)
        glT_ps = {}
        for c in (0, 1):
            glT_ps[c] = pspool.tile([128, FH], F32, tag="ps", name=f"glTps{c}")
        for k in range(KT):
            for c in (0, 1):
                nc.tensor.matmul(
                    glT_ps[c][:], wg_sb[:, k * 128:(k + 1) * 128],
                    xT[k][:, c * FH:(c + 1) * FH],
                    start=(k == 0), stop=(k == KT - 1),
                )
            if k < KT - 1:
                dummies(5)
        glT_sb = const.tile([NE, M], F32R, tag="glT")
        for c in (0, 1):
            nc.vector.tensor_copy(glT_sb[:, c * FH:(c + 1) * FH], glT_ps[c][0:NE, :])

        # ---- per token tile: logits transpose (as a plain matmul so the
        #      PE array stays active), masked softmax, gates transpose,
        #      bias-combine matmuls into the accumulator ----
        glp_t = []
        for i in range(IT):
            glp = pspool.tile([128, NE], F32, tag="ps", name=f"glp{i}")
            nc.tensor.matmul(
                glp[:], glT_sb[:, i * 128:(i + 1) * 128], identR[0:NE, 0:NE],
                start=True, stop=True,
            )
            glp_t.append(glp)
            dummies(2)
        gates_t = []
        for i in range(IT):
            glp = glp_t[i]
            negmax = smallp.tile([128, 1], F32, tag="s1")
            nc.vector.tensor_reduce(
                negmax[:], glp[:], axis=AX.X, op=ALU.max, negate=True
            )
            exps = smallp.tile([128, NE], F32, tag="s4")
            expsum = smallp.tile([128, 1], F32, tag="s1")
            nc.scalar.activation(
                exps[:], glp[:], ACTF.Exp,
                bias=negmax[:], scale=1.0, accum_out=expsum[:],
            )
            rinv = smallp.tile([128, 1], F32, tag="s1")
            nc.vector.reciprocal(rinv[:], expsum[:])
            rm = smallp.tile([128, 1], F32, tag="s1")
            nc.vector.tensor_mul(rm[:], rinv[:], mask_sb[:, i:i + 1])
            gates = gatep.tile([128, NE], F32R, tag="g")
            nc.vector.tensor_scalar_mul(gates[:], exps[:], rm[:])
            gates_t.append(gates)
        dummies(6)
        gTp = const.tile([128, 128], F32R, tag="gTp")
        nc.vector.tensor_copy(gTp[:], zsrc0[:, 0:128])  # rows 4+ stay zero
        for i in range(IT):
            gtp = pspool.tile([NE, 128], F32, tag="ps", name=f"gtp{i}")
            nc.tensor.matmul(
                gtp[:], gates_t[i][:], identR[:], start=True, stop=True
            )
            nc.vector.tensor_copy(gTp[0:NE, :], gtp[:])
            for c in (0, 1):
                bp = zpool.tile([128, FH], F32, tag="z", name=f"bps{i}{c}")
                nc.tensor.matmul(
                    bp[:], gTp[:], ball_sb[:, c * FH:(c + 1) * FH],
                    start=True, stop=True,
                )
                nc.scalar.copy(selA[i][:, c * FH:(c + 1) * FH], bp[:])
            dummies(2)
        dummies(6)

        # ---- expert matmul stream: half-pass (c) outer so each half-pass
        #      only needs 2.1MB of fresh weights -> stall-free stream start.
        #      LN tail split across the two e3 half-passes. ----
        bn6s = [None] * IT
        src_l, dst_l = selA, selB
        for e in range(NE):
            last = e == NE - 1
            for c in (0, 1):
                cs = slice(c * FH, (c + 1) * FH)
                for i in range(IT):
                    isl = slice(i * 128, (i + 1) * 128)
                    zt = zpool.tile([128, FH], F32, tag="z")
                    for k in range(KT):
                        nc.tensor.matmul(
                            zt[:], xT[k][:, isl], w_sb[(e, c, k)][:],
                            start=(k == 0), stop=(k == KT - 1),
                        )
                    if not last:
                        nc.vector.scalar_tensor_tensor(
                            dst_l[i][:, cs], zt[:], gates_t[i][:, e:e + 1],
                            src_l[i][:, cs], op0=ALU.mult, op1=ALU.add,
                        )
                        continue
                    nc.vector.scalar_tensor_tensor(
                        dst_l[i][:, cs], zt[:], gates_t[i][:, e:e + 1],
                        src_l[i][:, cs], op0=ALU.mult, op1=ALU.add,
                    )
                    if c == 0:
                        bn6 = smallp.tile([128, 2 * 6], F32, tag="bn6")
                        nc.vector.bn_stats(bn6[:, 0:6], dst_l[i][:, 0:FH])
                        bn6s[i] = bn6
                        continue
                    selF = dst_l[i]
                    bn6 = bn6s[i]
                    nc.vector.bn_stats(bn6[:, 6:12], selF[:, FH:D])
                    mv = smallp.tile([128, 2], F32, tag="mv")
                    nc.vector.bn_aggr(mv[:], bn6[:])
                    avi = smallp.tile([128, 1], F32, tag="s1")
                    nc.vector.tensor_scalar_add(avi[:], mv[:, 1:2], EPS)
                    sdi = smallp.tile([128, 1], F32, tag="s1")
                    nc.scalar.sqrt(sdi[:], avi[:])
                    ri = smallp.tile([128, 1], F32, tag="s1")
                    nc.vector.reciprocal(ri[:], sdi[:])
                    mbt = smallp.tile([128, 1], F32, tag="s1")
                    nc.vector.tensor_mul(mbt[:], mv[:, 0:1], ri[:])
                    mbi = smallp.tile([128, 1], F32, tag="s1")
                    nc.vector.tensor_scalar_mul(mbi[:], mbt[:], -1.0)
                    # ln = sel*rstd + mb on ACT, residual adds on GpSimd
                    xi = xtokp.tile([128, D], F32, tag="xtok")
                    nc.sync.dma_start(xi[:], xtok_d[i * 128:(i + 1) * 128, :])
                    lnb = scrp.tile([128, D], F32, tag="scr")
                    nc.scalar.activation(
                        lnb[:], selF[:], ACTF.Identity, bias=mbi[:], scale=ri[:]
                    )
                    if not trivial_affine:
                        lng = scrp.tile([128, D], F32, tag="scr")
                        nc.vector.tensor_mul(lng[:], lnb[:], gam_sb[:])
                        lnb2 = scrp.tile([128, D], F32, tag="scr")
                        nc.vector.tensor_add(lnb2[:], lng[:], bet_sb[:])
                        lnb = lnb2
                    outt = outp.tile([128, D], F32, tag="out")
                    nc.gpsimd.tensor_add(outt[:, 0:FH], lnb[:, 0:FH], xi[:, 0:FH])
                    nc.gpsimd.tensor_add(outt[:, FH:D], lnb[:, FH:D], xi[:, FH:D])
                    nc.sync.dma_start(out_d[i * 128:(i + 1) * 128, :], outt[:])
            src_l, dst_l = dst_l, src_l

    nc.finalize()
    return nc


_PROGRAM_CACHE: dict = {}


def _get_program(trivial_affine: bool) -> bass.Bass:
    key = trivial_affine
    if key not in _PROGRAM_CACHE:
        _PROGRAM_CACHE[key] = _build_program(trivial_affine)
    return _PROGRAM_CACHE[key]


def _pack_tokens(b: np.ndarray):
    """Partition 8192 tokens into 8 chunks of 1024, each chunk holding tokens
    of a single behavior (1..4) plus masked b==0 filler."""
    idx0 = np.flatnonzero(b == 0)
    chunks = []
    for t in range(1, NB + 1):
        idxs = np.flatnonzero(b == t)
        for s in range(0, max(len(idxs), 1), M):
            part = idxs[s:s + M]
            if len(part) or not chunks:
                chunks.append((part, t))
    chunks = [(p, t) for (p, t) in chunks if len(p) > 0]
    if len(chunks) > NCORES:
        raise RuntimeError(
            f"token packing needs {len(chunks)} single-behavior chunks > {NCORES}"
        )
    while len(chunks) < NCORES:
        chunks.append((np.empty((0,), np.int64), 1))
    p0 = 0
    cores = []
    for part, t in chunks:
        need = M - len(part)
        fill = idx0[p0:p0 + need]
        p0 += need
        if len(fill) != need:
            raise RuntimeError("not enough b==0 filler tokens for packing")
        idx = np.concatenate([part.astype(np.int64), fill.astype(np.int64)])
        msk = np.zeros((M,), np.float32)
        msk[:len(part)] = 1.0
        cores.append((idx, msk, t))
    assert p0 == len(idx0)
    return cores


def _behavior_tensors(W_sh, b_sh, W_sp, b_sp, w_gates):
    per_t = {}
    W_sh_flat = W_sh.reshape(NESH * D, D)
    for t in range(1, NB + 1):
        Wall = np.concatenate([W_sh_flat, W_sp[t - 1:t].reshape(D, D)], axis=0)
        wT = np.ascontiguousarray(Wall.T)                      # [D, NE*D]
        wt_h = np.ascontiguousarray(
            wT.reshape(KT, 128, NE, 2, FH).transpose(2, 3, 0, 1, 4)
        )                                                      # [e, c, k, 128, FH]
        wg_h = np.zeros((128, KT * 128), np.float32)
        wg_k = w_gates[t - 1].reshape(KT, 128, NE).transpose(1, 0, 2)  # [128, KT, NE]
        for k in range(KT):
            wg_h[:, k * 128:k * 128 + NE] = wg_k[:, k, :]
        ball_h = np.zeros((128, D), np.float32)
        ball_h[0:NE] = np.stack([b_sh[0], b_sh[1], b_sh[2], b_sp[t - 1]], axis=0)
        per_t[t] = (wt_h, wg_h, ball_h)
    return per_t


def _prepare(x, b_seq, W_sh, b_sh, W_sp, b_sp, w_gates, gamma, beta):
    x = np.ascontiguousarray(np.asarray(x, dtype=np.float32))
    b = np.asarray(b_seq).astype(np.int64).ravel()
    W_sh = np.asarray(W_sh, dtype=np.float32)
    b_sh = np.asarray(b_sh, dtype=np.float32)
    W_sp = np.asarray(W_sp, dtype=np.float32)
    b_sp = np.asarray(b_sp, dtype=np.float32)
    w_gates = np.asarray(w_gates, dtype=np.float32)
    gamma = np.asarray(gamma, dtype=np.float32)
    beta = np.asarray(beta, dtype=np.float32)
    assert x.shape == (N, D) and b.shape == (N,)

    trivial = bool(np.all(gamma == 1.0) and np.all(beta == 0.0))
    cores = _pack_tokens(b)
    per_t = _behavior_tensors(W_sh, b_sh, W_sp, b_sp, w_gates)

    in_maps = []
    for idx, msk, t in cores:
        wt_h, wg_h, ball_h = per_t[t]
        xc = np.ascontiguousarray(x[idx])                      # [M, D]
        xt_h = np.ascontiguousarray(xc.T).reshape(KT, 128, M)  # [k, 128, M]
        m = {
            "xt": xt_h,
            "xtok": xc,
            "wt": wt_h,
            "wg": wg_h,
            "ball": ball_h,
            "mask": np.ascontiguousarray(msk.reshape(IT, 128).T),
        }
        if not trivial:
            m["gam"] = np.ascontiguousarray(np.broadcast_to(gamma, (128, D)))
            m["bet"] = np.ascontiguousarray(np.broadcast_to(beta, (128, D)))
        in_maps.append(m)
    return trivial, cores, in_maps


def kernel_with_results(trace: bool = False, **inputs):
    trivial, cores, in_maps = _prepare(**inputs)
    nc = _get_program(trivial)
    res = run_bass_kernel_spmd(
        nc, in_maps, list(range(NCORES)), trace=trace
    )
    out = np.empty((N, D), np.float32)
    for c, (idx, _msk, _t) in enumerate(cores):
        out[idx] = res.results[c]["out"]
    return out, res


def kernel(**inputs) -> np.ndarray:
    out, _ = kernel_with_results(trace=False, **inputs)
    return out



# revision 2
# speedup vs baseline: 1.1929x; 1.1929x over previous
"""BehaviorMoE Trainium2 kernel (8 NeuronCores, SPMD data-parallel over sorted tokens).

Contract: kernel(**inputs) takes FULL inputs as returned by setup_inputs() and
returns the FULL [8192, 1024] float32 output.

Strategy (v2):
  - Host: sort tokens by behavior id. Each behavior t in 1..4 owns two cores;
    each core gets M=896 tokens (7x128 tiles) of that single behavior, padded
    with masked b==0 filler. Leftover b==0 tokens (out = x + beta) never touch
    the device and are assembled on host.
  - Device (identical SPMD program, per-core data), token-tile-pipelined:
      Phase 1 (k-paced by the xT DMAs): gate logits (PE, 2 psum banks) +
        expert-0 c0-half wave for tiles 0..5 (6 psum banks).
      Phase 1b: logits->SBUF, per-tile transpose + masked softmax, gates
        transpose, bias combine gates^T @ b_all (PE) copied into selA.
      e0 rest + e1: per (c, tile) 8 k-matmuls (PE, fp32r, N=512) + fused DVE
        scalar_tensor_tensor gate-weighted accumulate (ping-pong SBUF
        accumulators; in-place DVE ops fault on this HW).
      Fused e2+e3 pass per tile: both experts' 4 half-passes back-to-back,
        then immediately that tile's LayerNorm (bn_stats halves), normalize
        (ACT), residual add (GpSimd, one full-D op), DMA out -- so the tail
        of each tile hides under the next tile's matmul stream.
  - Host: scatter per-core outputs back to original token order.
"""

import os
import sys

import numpy as np

for _p in ("/opt/trn_rl_repo", "/root/.axon_site/_ro/trn_rl_repo"):
    if os.path.isdir(_p) and _p not in sys.path:
        sys.path.append(_p)

from contextlib import ExitStack

from concourse import bacc, bass, masks, mybir, tile
from concourse.bass_utils import run_bass_kernel_spmd

F32 = mybir.dt.float32
F32R = mybir.dt.float32r
AX = mybir.AxisListType
ALU = mybir.AluOpType
ACTF = mybir.ActivationFunctionType

D = 1024            # model dim
N = 8192            # tokens
NB = 4              # behaviors
NESH = 3            # shared experts
NE = 4              # experts per behavior (3 shared + 1 specific)
EPS = 1e-5
NCORES = 8
M = 896             # tokens per core (7 tiles of 128)
KT = D // 128       # k tiles (contraction)
IT = M // 128       # token tiles per core
FH = 512            # feature half-tile (psum bank width in f32)
C1 = M - FH         # second logit token chunk (384)


def _build_program(trivial_affine: bool) -> bass.Bass:
    nc = bacc.Bacc()

    xt_d = nc.declare_dram_parameter("xt", [KT, 128, M], F32R, isOutput=False)
    xtok_d = nc.declare_dram_parameter("xtok", [M, D], F32, isOutput=False)
    wt_d = nc.declare_dram_parameter("wt", [NE, 2, KT, 128, FH], F32R, isOutput=False)
    wg_d = nc.declare_dram_parameter("wg", [128, KT * 128], F32R, isOutput=False)
    ball_d = nc.declare_dram_parameter("ball", [128, D], F32R, isOutput=False)
    mask_d = nc.declare_dram_parameter("mask", [128, IT], F32, isOutput=False)
    if not trivial_affine:
        gam_d = nc.declare_dram_parameter("gam", [128, D], F32, isOutput=False)
        bet_d = nc.declare_dram_parameter("bet", [128, D], F32, isOutput=False)
    out_d = nc.declare_dram_parameter("out", [M, D], F32, isOutput=True)

    with tile.TileContext(nc) as tc, ExitStack() as ctx:
        const = ctx.enter_context(tc.tile_pool(name="const", bufs=1))
        xtp = ctx.enter_context(tc.tile_pool(name="xt", bufs=KT))
        wpool = ctx.enter_context(tc.tile_pool(name="w", bufs=32))
        selp = ctx.enter_context(tc.tile_pool(name="sel", bufs=2 * IT))
        xtokp = ctx.enter_context(tc.tile_pool(name="xtok", bufs=3))
        outp = ctx.enter_context(tc.tile_pool(name="outp", bufs=2))
        scrp = ctx.enter_context(tc.tile_pool(name="scr", bufs=3))
        gatep = ctx.enter_context(tc.tile_pool(name="gate", bufs=IT))
        gtsp = ctx.enter_context(tc.tile_pool(name="gts", bufs=2))
        smallp = ctx.enter_context(tc.tile_pool(name="small", bufs=40))
        zpool = ctx.enter_context(tc.tile_pool(name="z", bufs=6, space="PSUM"))
        pspool = ctx.enter_context(tc.tile_pool(name="ps", bufs=2, space="PSUM"))

        # ---- small constant inputs ----
        wg_sb = const.tile([128, KT * 128], F32R, tag="wg")
        nc.sync.dma_start(wg_sb[:], wg_d[:])
        ball_sb = const.tile([128, D], F32R, tag="ball")
        nc.sync.dma_start(ball_sb[:], ball_d[:])
        mask_sb = const.tile([128, IT], F32, tag="mask")
        nc.sync.dma_start(mask_sb[:], mask_d[:])
        if not trivial_affine:
            gam_sb = const.tile([128, D], F32, tag="gam")
            nc.sync.dma_start(gam_sb[:], gam_d[:])
            bet_sb = const.tile([128, D], F32, tag="bet")
            nc.sync.dma_start(bet_sb[:], bet_d[:])

        # ---- resident xT k-tiles first, then streamed weight half-tiles ----
        xT = []
        for k in range(KT):
            t = xtp.tile([128, M], F32R, tag="xt")
            nc.sync.dma_start(t[:], xt_d[k])
            xT.append(t)
        w_sb = {}
        for e in range(NE):
            for c in (0, 1):
                for k in range(KT):
                    t = wpool.tile([128, FH], F32R, tag="w", name=f"w{e}{c}{k}")
                    nc.sync.dma_start(t[:], wt_d[e, c, k])
                    w_sb[(e, c, k)] = t

        identity = const.tile([128, 128], F32, tag="ident")
        masks.make_identity(nc, identity[:])
        identR = const.tile([128, 128], F32R, tag="identR")
        nc.vector.tensor_copy(identR[:], identity[:])

        # ---- accumulators (ping-pong; in-place DVE ops fault) ----
        selA = [selp.tile([128, D], F32, tag="sel", name=f"selA{i}") for i in range(IT)]
        selB = [selp.tile([128, D], F32, tag="sel", name=f"selB{i}") for i in range(IT)]

        def isl(i):
            return slice(i * 128, (i + 1) * 128)

        def csl(c):
            return slice(c * FH, (c + 1) * FH)

        def expert_mms(zt, e, c, i):
            for k in range(KT):
                nc.tensor.matmul(
                    zt[:], xT[k][:, isl(i)], w_sb[(e, c, k)][:],
                    start=(k == 0), stop=(k == KT - 1),
                )

        # ---- Phase 1: gate logits + e0/c0 wave for tiles 0..5, k-paced ----
        glc0 = pspool.tile([128, FH], F32, tag="ps", name="glc0")
        glc1 = pspool.tile([128, FH], F32, tag="ps", name="glc1")
        zt_w = [zpool.tile([128, FH], F32, tag="z", name=f"zw{i}") for i in range(6)]
        for k in range(KT):
            nc.tensor.matmul(
                glc0[:], wg_sb[:, isl(k)], xT[k][:, 0:FH],
                start=(k == 0), stop=(k == KT - 1),
            )
            nc.tensor.matmul(
                glc1[:, 0:C1], wg_sb[:, isl(k)], xT[k][:, FH:M],
                start=(k == 0), stop=(k == KT - 1),
            )
            for i in range(6):
                nc.tensor.matmul(
                    zt_w[i][:], xT[k][:, isl(i)], w_sb[(0, 0, k)][:],
                    start=(k == 0), stop=(k == KT - 1),
                )

        # ---- Phase 1b: logits -> SBUF, per-tile softmax, bias combine ----
        glT_sb = const.tile([NE, M], F32R, tag="glT")
        nc.vector.tensor_copy(glT_sb[:, 0:FH], glc0[0:NE, :])
        nc.vector.tensor_copy(glT_sb[:, FH:M], glc1[0:NE, 0:C1])

        gates_t = []
        for i in range(IT):
            glp = pspool.tile([128, FH], F32, tag="ps", name=f"glp{i}")
            nc.tensor.matmul(
                glp[:, 0:NE], glT_sb[:, isl(i)], identR[0:NE, 0:NE],
                start=True, stop=True,
            )
            negmax = smallp.tile([128, 1], F32, tag="s1")
            nc.vector.tensor_reduce(
                negmax[:], glp[:, 0:NE], axis=AX.X, op=ALU.max, negate=True
            )
            exps = smallp.tile([128, NE], F32, tag="s4")
            expsum = smallp.tile([128, 1], F32, tag="s1")
            nc.scalar.activation(
                exps[:], glp[:, 0:NE], ACTF.Exp,
                bias=negmax[:], scale=1.0, accum_out=expsum[:],
            )
            rinv = smallp.tile([128, 1], F32, tag="s1")
            nc.vector.reciprocal(rinv[:], expsum[:])
            rm = smallp.tile([128, 1], F32, tag="s1")
            nc.vector.tensor_mul(rm[:], rinv[:], mask_sb[:, i:i + 1])
            gates = gatep.tile([128, NE], F32R, tag="g")
            nc.vector.tensor_scalar_mul(gates[:], exps[:], rm[:])
            gates_t.append(gates)

        # gates transpose + bias combine into selA
        for i in range(IT):
            gtp = pspool.tile([128, FH], F32, tag="ps", name=f"gtp{i}")
            nc.tensor.matmul(
                gtp[0:NE, 0:128], gates_t[i][:], identR[:], start=True, stop=True
            )
            gT_sb = gtsp.tile([NE, 128], F32R, tag="gT")
            nc.vector.tensor_copy(gT_sb[:], gtp[0:NE, 0:128])
            for c in (0, 1):
                bp = pspool.tile([128, FH], F32, tag="ps", name=f"bp{i}{c}")
                nc.tensor.matmul(
                    bp[:], gT_sb[:], ball_sb[0:NE, csl(c)],
                    start=True, stop=True,
                )
                nc.scalar.copy(selA[i][:, csl(c)], bp[:])

        # ---- e0 ----
        for i in range(6):
            nc.vector.scalar_tensor_tensor(
                selB[i][:, csl(0)], zt_w[i][:], gates_t[i][:, 0:1],
                selA[i][:, csl(0)], op0=ALU.mult, op1=ALU.add,
            )
        zt6 = zpool.tile([128, FH], F32, tag="z", name="zw6")
        expert_mms(zt6, 0, 0, 6)
        nc.vector.scalar_tensor_tensor(
            selB[6][:, csl(0)], zt6[:], gates_t[6][:, 0:1],
            selA[6][:, csl(0)], op0=ALU.mult, op1=ALU.add,
        )
        for i in range(IT):
            zt = zpool.tile([128, FH], F32, tag="z")
            expert_mms(zt, 0, 1, i)
            nc.vector.scalar_tensor_tensor(
                selB[i][:, csl(1)], zt[:], gates_t[i][:, 0:1],
                selA[i][:, csl(1)], op0=ALU.mult, op1=ALU.add,
            )

        # ---- e1 ----
        for c in (0, 1):
            for i in range(IT):
                zt = zpool.tile([128, FH], F32, tag="z")
                expert_mms(zt, 1, c, i)
                nc.vector.scalar_tensor_tensor(
                    selA[i][:, csl(c)], zt[:], gates_t[i][:, 1:2],
                    selB[i][:, csl(c)], op0=ALU.mult, op1=ALU.add,
                )

        # ---- fused e2+e3 passes with per-tile LN tail ----
        for i in range(IT):
            bn6 = smallp.tile([128, 2 * 6], F32, tag="bn6")
            for c in (0, 1):
                zt2 = zpool.tile([128, FH], F32, tag="z")
                expert_mms(zt2, 2, c, i)
                nc.vector.scalar_tensor_tensor(
                    selB[i][:, csl(c)], zt2[:], gates_t[i][:, 2:3],
                    selA[i][:, csl(c)], op0=ALU.mult, op1=ALU.add,
                )
                zt3 = zpool.tile([128, FH], F32, tag="z")
                expert_mms(zt3, 3, c, i)
                nc.vector.scalar_tensor_tensor(
                    selA[i][:, csl(c)], zt3[:], gates_t[i][:, 3:4],
                    selB[i][:, csl(c)], op0=ALU.mult, op1=ALU.add,
                )
                nc.vector.bn_stats(bn6[:, 6 * c:6 * c + 6], selA[i][:, csl(c)])
            selF = selA[i]
            mv = smallp.tile([128, 2], F32, tag="mv")
            nc.vector.bn_aggr(mv[:], bn6[:])
            avi = smallp.tile([128, 1], F32, tag="s1")
            nc.vector.tensor_scalar_add(avi[:], mv[:, 1:2], EPS)
            sdi = smallp.tile([128, 1], F32, tag="s1")
            nc.scalar.sqrt(sdi[:], avi[:])
            ri = smallp.tile([128, 1], F32, tag="s1")
            nc.vector.reciprocal(ri[:], sdi[:])
            mbt = smallp.tile([128, 1], F32, tag="s1")
            nc.vector.tensor_mul(mbt[:], mv[:, 0:1], ri[:])
            mbi = smallp.tile([128, 1], F32, tag="s1")
            nc.vector.tensor_scalar_mul(mbi[:], mbt[:], -1.0)
            # ln = sel*rstd + mb on ACT, residual add on GpSimd
            xi = xtokp.tile([128, D], F32, tag="xtok")
            nc.sync.dma_start(xi[:], xtok_d[isl(i), :])
            lnb = scrp.tile([128, D], F32, tag="scr")
            nc.scalar.activation(
                lnb[:], selF[:], ACTF.Identity, bias=mbi[:], scale=ri[:]
            )
            if not trivial_affine:
                lng = scrp.tile([128, D], F32, tag="scr")
                nc.vector.tensor_mul(lng[:], lnb[:], gam_sb[:])
                lnb2 = scrp.tile([128, D], F32, tag="scr")
                nc.vector.tensor_add(lnb2[:], lng[:], bet_sb[:])
                lnb = lnb2
            outt = outp.tile([128, D], F32, tag="out")
            nc.gpsimd.tensor_add(outt[:], lnb[:], xi[:])
            nc.sync.dma_start(out_d[isl(i), :], outt[:])

    nc.finalize()
    return nc


_PROGRAM_CACHE: dict = {}


def _get_program(trivial_affine: bool) -> bass.Bass:
    key = trivial_affine
    if key not in _PROGRAM_CACHE:
        _PROGRAM_CACHE[key] = _build_program(trivial_affine)
    return _PROGRAM_CACHE[key]


def _pack_tokens(b: np.ndarray):
    """Two cores per behavior t in 1..4, M=896 tokens each, padded with masked
    b==0 filler. Returns (per-core (idx, mask, t) list, leftover b==0 idx)."""
    idx0 = np.flatnonzero(b == 0)
    p0 = 0
    cores = []
    for t in range(1, NB + 1):
        idxs = np.flatnonzero(b == t)
        if len(idxs) > 2 * M:
            raise RuntimeError(
                f"behavior {t} has {len(idxs)} tokens > capacity {2 * M}"
            )
        for s in (0, M):
            part = idxs[s:s + M]
            need = M - len(part)
            fill = idx0[p0:p0 + need]
            p0 += need
            if len(fill) != need:
                raise RuntimeError("not enough b==0 filler tokens for packing")
            idx = np.concatenate([part.astype(np.int64), fill.astype(np.int64)])
            msk = np.zeros((M,), np.float32)
            msk[:len(part)] = 1.0
            cores.append((idx, msk, t))
    return cores, idx0[p0:]


def _behavior_tensors(W_sh, b_sh, W_sp, b_sp, w_gates):
    per_t = {}
    W_sh_flat = W_sh.reshape(NESH * D, D)
    for t in range(1, NB + 1):
        Wall = np.concatenate([W_sh_flat, W_sp[t - 1:t].reshape(D, D)], axis=0)
        wT = np.ascontiguousarray(Wall.T)                      # [D, NE*D]
        wt_h = np.ascontiguousarray(
            wT.reshape(KT, 128, NE, 2, FH).transpose(2, 3, 0, 1, 4)
        )                                                      # [e, c, k, 128, FH]
        wg_h = np.zeros((128, KT * 128), np.float32)
        wg_k = w_gates[t - 1].reshape(KT, 128, NE).transpose(1, 0, 2)  # [128, KT, NE]
        for k in range(KT):
            wg_h[:, k * 128:k * 128 + NE] = wg_k[:, k, :]
        ball_h = np.zeros((128, D), np.float32)
        ball_h[0:NE] = np.stack([b_sh[0], b_sh[1], b_sh[2], b_sp[t - 1]], axis=0)
        per_t[t] = (wt_h, wg_h, ball_h)
    return per_t


def _prepare(x, b_seq, W_sh, b_sh, W_sp, b_sp, w_gates, gamma, beta):
    x = np.ascontiguousarray(np.asarray(x, dtype=np.float32))
    b = np.asarray(b_seq).astype(np.int64).ravel()
    W_sh = np.asarray(W_sh, dtype=np.float32)
    b_sh = np.asarray(b_sh, dtype=np.float32)
    W_sp = np.asarray(W_sp, dtype=np.float32)
    b_sp = np.asarray(b_sp, dtype=np.float32)
    w_gates = np.asarray(w_gates, dtype=np.float32)
    gamma = np.asarray(gamma, dtype=np.float32)
    beta = np.asarray(beta, dtype=np.float32)
    assert x.shape == (N, D) and b.shape == (N,)

    trivial = bool(np.all(gamma == 1.0) and np.all(beta == 0.0))
    cores, leftover = _pack_tokens(b)
    per_t = _behavior_tensors(W_sh, b_sh, W_sp, b_sp, w_gates)

    in_maps = []
    for idx, msk, t in cores:
        wt_h, wg_h, ball_h = per_t[t]
        xc = np.ascontiguousarray(x[idx])                      # [M, D]
        xt_h = np.ascontiguousarray(xc.T).reshape(KT, 128, M)  # [k, 128, M]
        m = {
            "xt": xt_h,
            "xtok": xc,
            "wt": wt_h,
            "wg": wg_h,
            "ball": ball_h,
            "mask": np.ascontiguousarray(msk.reshape(IT, 128).T),
        }
        if not trivial:
            m["gam"] = np.ascontiguousarray(np.broadcast_to(gamma, (128, D)))
            m["bet"] = np.ascontiguousarray(np.broadcast_to(beta, (128, D)))
        in_maps.append(m)
    return trivial, cores, leftover, in_maps, x, beta


def kernel_with_results(trace: bool = False, **inputs):
    trivial, cores, leftover, in_maps, x, beta = _prepare(**inputs)
    nc = _get_program(trivial)
    res = run_bass_kernel_spmd(
        nc, in_maps, list(range(NCORES)), trace=trace
    )
    out = np.empty((N, D), np.float32)
    for c, (idx, _msk, _t) in enumerate(cores):
        out[idx] = res.results[c]["out"]
    if len(leftover):
        out[leftover] = x[leftover] + beta[None, :]
    return out, res


def kernel(**inputs) -> np.ndarray:
    out, _ = kernel_with_results(trace=False, **inputs)
    return out


# revision 10
# speedup vs baseline: 1.2037x; 1.0090x over previous
"""BehaviorMoE Trainium2 kernel (8 NeuronCores, SPMD data-parallel over sorted tokens).

Contract: kernel(**inputs) takes FULL inputs as returned by setup_inputs() and
returns the FULL [8192, 1024] float32 output.

Strategy (v3):
  - Host: sort tokens by behavior id. Each behavior t in 1..4 owns two cores;
    each core gets M=896 tokens (7x128 tiles) of that single behavior, padded
    with masked b==0 filler. Leftover b==0 tokens (out = x + beta) never touch
    the device and are assembled on host.
  - Device (identical SPMD program, per-core data):
      Expert matmuls run in bf16 (x and W bf16; ~0.4% rel err on expert
      outputs, well within tolerance); gate logits run in fp32r off a
      separate f32 copy of xT because softmax amplifies logit rounding.
      All weights stay resident in SBUF (8MB bf16); DMA issues are batched
      (one descriptor per weight half-matrix) and spread across the Sync/
      Scalar/GpSimd engines so the PE's first matmul lands early.
      Combine chain per (tile, feature-half): e0 gate-scale (tensor_scalar,
      no bias dependency -> PSUM banks recycle immediately), e1..e3 fused
      scalar_tensor_tensor accumulates (ping-pong bf16 SBUF accumulators;
      in-place DVE ops fault), then the gate-combined bias (tiny PE matmul
      gates^T @ b_all issued in the fused pass) is added straight out of
      PSUM by a DVE tensor_tensor.
      e2+e3 run fused per tile so each tile's LayerNorm (bn_stats halves),
      normalize (ACT), residual (split Vector/GpSimd) and output DMA hide
      under the next tile's matmul stream.
  - Host: scatter per-core outputs back to original token order.
"""

import os
import sys

import numpy as np
import ml_dtypes

for _p in ("/opt/trn_rl_repo", "/root/.axon_site/_ro/trn_rl_repo"):
    if os.path.isdir(_p) and _p not in sys.path:
        sys.path.append(_p)

from contextlib import ExitStack

from concourse import bacc, bass, masks, mybir, tile
from concourse.bass_utils import run_bass_kernel_spmd

F32 = mybir.dt.float32
F32R = mybir.dt.float32r
BF16 = mybir.dt.bfloat16
AX = mybir.AxisListType
ALU = mybir.AluOpType
ACTF = mybir.ActivationFunctionType

D = 1024            # model dim
N = 8192            # tokens
NB = 4              # behaviors
NESH = 3            # shared experts
NE = 4              # experts per behavior (3 shared + 1 specific)
EPS = 1e-5
NCORES = 8
M = 896             # tokens per core (7 tiles of 128)
KT = D // 128       # k tiles (contraction)
IT = M // 128       # token tiles per core
FH = 512            # feature half-tile (psum bank width in f32)
C1 = M - FH         # second logit token chunk (384)


def _build_program(trivial_affine: bool) -> bass.Bass:
    nc = bacc.Bacc()

    xt_d = nc.declare_dram_parameter("xt", [128, KT * M], F32R, isOutput=False)
    xtb_d = nc.declare_dram_parameter("xtb", [128, KT * M], BF16, isOutput=False)
    xtok_d = nc.declare_dram_parameter("xtok", [M, D], F32, isOutput=False)
    wt_d = nc.declare_dram_parameter("wt", [NE, 2, 128, KT * FH], BF16, isOutput=False)
    wg_d = nc.declare_dram_parameter("wg", [128, KT * 128], F32R, isOutput=False)
    ball_d = nc.declare_dram_parameter("ball", [NE, D], F32R, isOutput=False)
    mask_d = nc.declare_dram_parameter("mask", [128, IT], F32, isOutput=False)
    if not trivial_affine:
        gam_d = nc.declare_dram_parameter("gam", [128, D], F32, isOutput=False)
        bet_d = nc.declare_dram_parameter("bet", [128, D], F32, isOutput=False)
    out_d = nc.declare_dram_parameter("out", [M, D], F32, isOutput=True)

    with tile.TileContext(nc) as tc, ExitStack() as ctx:
        const = ctx.enter_context(tc.tile_pool(name="const", bufs=1))
        wpool = ctx.enter_context(tc.tile_pool(name="w", bufs=2 * NE))
        selp = ctx.enter_context(tc.tile_pool(name="sel", bufs=2 * IT))
        xtokp = ctx.enter_context(tc.tile_pool(name="xtok", bufs=IT))
        outp = ctx.enter_context(tc.tile_pool(name="outp", bufs=3))
        scrp = ctx.enter_context(tc.tile_pool(name="scr", bufs=3))
        gatep = ctx.enter_context(tc.tile_pool(name="gate", bufs=IT))
        gtsp = ctx.enter_context(tc.tile_pool(name="gts", bufs=IT))
        smallp = ctx.enter_context(tc.tile_pool(name="small", bufs=40))
        zpool = ctx.enter_context(tc.tile_pool(name="z", bufs=6, space="PSUM"))
        pspool = ctx.enter_context(tc.tile_pool(name="ps", bufs=2, space="PSUM"))

        # ---- DMA issues, spread across engines so they all start early ----
        # sync: gate weights, bf16 xT, all expert weights
        wg_sb = const.tile([128, KT * 128], F32R, tag="wg")
        nc.sync.dma_start(wg_sb[:], wg_d[:])
        xtb = const.tile([128, KT * M], BF16, tag="xtb")
        half = KT * M // 2
        nc.sync.dma_start(xtb[:, 0:half], xtb_d[:, 0:half])
        w_sb = {}
        for e in range(NE):
            for c in (0, 1):
                w_sb[(e, c)] = wpool.tile(
                    [128, KT * FH], BF16, tag="w", name=f"w{e}{c}"
                )
        nc.sync.dma_start(w_sb[(0, 0)][:, 0:4 * FH], wt_d[0, 0, :, 0:4 * FH])
        nc.sync.dma_start(xtb[:, half:], xtb_d[:, half:])
        nc.sync.dma_start(w_sb[(0, 0)][:, 4 * FH:], wt_d[0, 0, :, 4 * FH:])
        nc.sync.dma_start(w_sb[(0, 1)][:], wt_d[0, 1])
        for e in range(1, NE):
            for c in (0, 1):
                nc.sync.dma_start(w_sb[(e, c)][:], wt_d[e, c])

        # scalar: f32 xT (for gate logits; can trail), small consts
        xT = const.tile([128, KT * M], F32R, tag="xt")
        nc.scalar.dma_start(xT[:, 0:half], xt_d[:, 0:half])
        nc.scalar.dma_start(xT[:, half:], xt_d[:, half:])
        ball_sb = const.tile([NE, D], F32R, tag="ball")
        nc.scalar.dma_start(ball_sb[:], ball_d[:])
        mask_sb = const.tile([128, IT], F32, tag="mask")
        nc.scalar.dma_start(mask_sb[:], mask_d[:])
        if not trivial_affine:
            gam_sb = const.tile([128, D], F32, tag="gam")
            nc.scalar.dma_start(gam_sb[:], gam_d[:])
            bet_sb = const.tile([128, D], F32, tag="bet")
            nc.scalar.dma_start(bet_sb[:], bet_d[:])

        # gpsimd: residual-input tokens (needed late)
        xtok_t = []
        for i in range(IT):
            xi = xtokp.tile([128, D], F32, tag="xtok", name=f"xi{i}")
            nc.gpsimd.dma_start(xi[:], xtok_d[i * 128:(i + 1) * 128, :])
            xtok_t.append(xi)

        identity = const.tile([128, 128], F32, tag="ident")
        masks.make_identity(nc, identity[:])
        identR = const.tile([128, 128], F32R, tag="identR")
        nc.vector.tensor_copy(identR[:], identity[:])

        # ---- accumulators (ping-pong; in-place DVE ops fault) ----
        selA = [selp.tile([128, D], BF16, tag="sel", name=f"selA{i}")
                for i in range(IT)]
        selB = [selp.tile([128, D], BF16, tag="sel", name=f"selB{i}")
                for i in range(IT)]

        def isl(i):
            return slice(i * 128, (i + 1) * 128)

        def csl(c):
            return slice(c * FH, (c + 1) * FH)

        def expert_group(e, c, i):
            zt = zpool.tile([128, FH], F32, tag="z")
            for k in range(KT):
                nc.tensor.matmul(
                    zt[:], xtb[:, k * M + i * 128:k * M + (i + 1) * 128],
                    w_sb[(e, c)][:, k * FH:(k + 1) * FH],
                    start=(k == 0), stop=(k == KT - 1),
                )
            return zt

        # ---- e0/c0 with gate logits interleaved ----
        glc0 = pspool.tile([128, FH], F32, tag="ps", name="glc0")
        glc1 = pspool.tile([128, FH], F32, tag="ps", name="glc1")

        def logit_mms(krange):
            for k in krange:
                nc.tensor.matmul(
                    glc0[:], wg_sb[:, isl(k)], xT[:, k * M:k * M + FH],
                    start=(k == 0), stop=(k == KT - 1),
                )
                nc.tensor.matmul(
                    glc1[:, 0:C1], wg_sb[:, isl(k)], xT[:, k * M + FH:(k + 1) * M],
                    start=(k == 0), stop=(k == KT - 1),
                )

        zt_e0c0 = []
        for i in range(4):
            zt_e0c0.append(expert_group(0, 0, i))
            if i == 1:
                logit_mms(range(0, 4))
            elif i == 3:
                logit_mms(range(4, KT))

        # logits -> SBUF, per-tile transpose + masked softmax + gates^T
        glT_sb = const.tile([NE, M], F32R, tag="glT")
        nc.vector.tensor_copy(glT_sb[:, 0:FH], glc0[0:NE, :])
        nc.vector.tensor_copy(glT_sb[:, FH:M], glc1[0:NE, 0:C1])

        gates_t = []
        for i in range(IT):
            glp = pspool.tile([128, FH], F32, tag="ps", name=f"glp{i}")
            nc.tensor.matmul(
                glp[:, 0:NE], glT_sb[:, isl(i)], identR[0:NE, 0:NE],
                start=True, stop=True,
            )
            negmax = smallp.tile([128, 1], F32, tag="s1")
            nc.vector.tensor_reduce(
                negmax[:], glp[:, 0:NE], axis=AX.X, op=ALU.max, negate=True
            )
            exps = smallp.tile([128, NE], F32, tag="s4")
            expsum = smallp.tile([128, 1], F32, tag="s1")
            nc.scalar.activation(
                exps[:], glp[:, 0:NE], ACTF.Exp,
                bias=negmax[:], scale=1.0, accum_out=expsum[:],
            )
            rinv = smallp.tile([128, 1], F32, tag="s1")
            nc.vector.reciprocal(rinv[:], expsum[:])
            rm = smallp.tile([128, 1], F32, tag="s1")
            nc.vector.tensor_mul(rm[:], rinv[:], mask_sb[:, i:i + 1])
            gates = gatep.tile([128, NE], F32, tag="g")
            nc.vector.tensor_scalar_mul(gates[:], exps[:], rm[:])
            gates_t.append(gates)

        # remaining e0/c0 groups interleaved with the gates^T transposes
        # (gtp(i) waits on softmax(i) on Vector, so give the PE dense work)
        gT_t = []

        def gtp_block(irange):
            for i in irange:
                gtp = pspool.tile([128, FH], F32, tag="ps", name=f"gtp{i}")
                nc.tensor.matmul(
                    gtp[0:NE, 0:128], gates_t[i][:], identity[:],
                    start=True, stop=True,
                )
                gT_sb = gtsp.tile([NE, 128], F32R, tag="gT", name=f"gT{i}")
                nc.vector.tensor_copy(gT_sb[:], gtp[0:NE, 0:128])
                gT_t.append(gT_sb)

        zt_e0c0.append(expert_group(0, 0, 4))
        gtp_block(range(0, 4))
        zt_e0c0.append(expert_group(0, 0, 5))
        gtp_block(range(4, IT))
        zt_e0c0.append(expert_group(0, 0, 6))
        # e0/c0 combines: plain gate-scale, PSUM banks recycle as gates land
        for i in range(IT):
            nc.vector.tensor_scalar_mul(
                selB[i][:, csl(0)], zt_e0c0[i][:], gates_t[i][:, 0:1]
            )

        # ---- e0/c1 and e1 ----
        for i in range(IT):
            zt = expert_group(0, 1, i)
            nc.vector.tensor_scalar_mul(
                selB[i][:, csl(1)], zt[:], gates_t[i][:, 0:1]
            )
        for c in (0, 1):
            for i in range(IT):
                zt = expert_group(1, c, i)
                nc.vector.scalar_tensor_tensor(
                    selA[i][:, csl(c)], zt[:], gates_t[i][:, 1:2],
                    selB[i][:, csl(c)], op0=ALU.mult, op1=ALU.add,
                )

        # ---- fused e2+e3 passes with bias join and per-tile LN tail ----
        for i in range(IT):
            bn6 = smallp.tile([128, 2 * 6], F32, tag="bn6")
            bp = {}
            for c in (0, 1):
                bp[c] = pspool.tile([128, FH], F32, tag="ps", name=f"bp{i}{c}")
                nc.tensor.matmul(
                    bp[c][:], gT_t[i][:], ball_sb[:, csl(c)],
                    start=True, stop=True,
                )
            for c in (0, 1):
                zt2 = expert_group(2, c, i)
                nc.vector.scalar_tensor_tensor(
                    selB[i][:, csl(c)], zt2[:], gates_t[i][:, 2:3],
                    selA[i][:, csl(c)], op0=ALU.mult, op1=ALU.add,
                )
                zt3 = expert_group(3, c, i)
                nc.vector.scalar_tensor_tensor(
                    selA[i][:, csl(c)], zt3[:], gates_t[i][:, 3:4],
                    selB[i][:, csl(c)], op0=ALU.mult, op1=ALU.add,
                )
                nc.vector.tensor_add(
                    selB[i][:, csl(c)], bp[c][:], selA[i][:, csl(c)]
                )
                nc.vector.bn_stats(bn6[:, 6 * c:6 * c + 6], selB[i][:, csl(c)])
            selF = selB[i]
            mv = smallp.tile([128, 2], F32, tag="mv")
            nc.vector.bn_aggr(mv[:], bn6[:])
            avi = smallp.tile([128, 1], F32, tag="s1")
            nc.vector.tensor_scalar_add(avi[:], mv[:, 1:2], EPS)
            sdi = smallp.tile([128, 1], F32, tag="s1")
            nc.scalar.sqrt(sdi[:], avi[:])
            ri = smallp.tile([128, 1], F32, tag="s1")
            nc.vector.reciprocal(ri[:], sdi[:])
            mbt = smallp.tile([128, 1], F32, tag="s1")
            nc.vector.tensor_mul(mbt[:], mv[:, 0:1], ri[:])
            mbi = smallp.tile([128, 1], F32, tag="s1")
            nc.vector.tensor_scalar_mul(mbi[:], mbt[:], -1.0)
            # ln = sel*rstd + mb on ACT (halves), residual split Vector/GpSimd
            xi = xtok_t[i]
            outt = outp.tile([128, D], F32, tag="out")
            for c in (0, 1):
                lnb = scrp.tile([128, FH], F32, tag="scr")
                nc.scalar.activation(
                    lnb[:], selF[:, csl(c)], ACTF.Identity,
                    bias=mbi[:], scale=ri[:],
                )
                if not trivial_affine:
                    lng = scrp.tile([128, FH], F32, tag="scr")
                    nc.vector.tensor_mul(lng[:], lnb[:], gam_sb[:, csl(c)])
                    lnb2 = scrp.tile([128, FH], F32, tag="scr")
                    nc.vector.tensor_add(lnb2[:], lng[:], bet_sb[:, csl(c)])
                    lnb = lnb2
                eng = nc.gpsimd if c == 0 else nc.vector
                eng.tensor_add(outt[:, csl(c)], lnb[:], xi[:, csl(c)])
                nc.sync.dma_start(out_d[isl(i), csl(c)], outt[:, csl(c)])

    nc.finalize()
    return nc


_PROGRAM_CACHE: dict = {}


def _get_program(trivial_affine: bool) -> bass.Bass:
    key = trivial_affine
    if key not in _PROGRAM_CACHE:
        _PROGRAM_CACHE[key] = _build_program(trivial_affine)
    return _PROGRAM_CACHE[key]


def _pack_tokens(b: np.ndarray):
    """Two cores per behavior t in 1..4, M=896 tokens each, padded with masked
    b==0 filler. Returns (per-core (idx, mask, t) list, leftover b==0 idx)."""
    idx0 = np.flatnonzero(b == 0)
    p0 = 0
    cores = []
    for t in range(1, NB + 1):
        idxs = np.flatnonzero(b == t)
        if len(idxs) > 2 * M:
            raise RuntimeError(
                f"behavior {t} has {len(idxs)} tokens > capacity {2 * M}"
            )
        for s in (0, M):
            part = idxs[s:s + M]
            need = M - len(part)
            fill = idx0[p0:p0 + need]
            p0 += need
            if len(fill) != need:
                raise RuntimeError("not enough b==0 filler tokens for packing")
            idx = np.concatenate([part.astype(np.int64), fill.astype(np.int64)])
            msk = np.zeros((M,), np.float32)
            msk[:len(part)] = 1.0
            cores.append((idx, msk, t))
    return cores, idx0[p0:]


def _behavior_tensors(W_sh, b_sh, W_sp, b_sp, w_gates):
    per_t = {}
    W_sh_flat = W_sh.reshape(NESH * D, D)
    for t in range(1, NB + 1):
        Wall = np.concatenate([W_sh_flat, W_sp[t - 1:t].reshape(D, D)], axis=0)
        wT = np.ascontiguousarray(Wall.T)                      # [D, NE*D]
        # [e, c, p, k*FH + f] = wT[128k + p, e*D + c*FH + f]
        wt_h = np.ascontiguousarray(
            wT.reshape(KT, 128, NE, 2, FH).transpose(2, 3, 1, 0, 4)
            .reshape(NE, 2, 128, KT * FH).astype(ml_dtypes.bfloat16)
        )
        wg_h = np.zeros((128, KT * 128), np.float32)
        wg_k = w_gates[t - 1].reshape(KT, 128, NE).transpose(1, 0, 2)  # [128, KT, NE]
        for k in range(KT):
            wg_h[:, k * 128:k * 128 + NE] = wg_k[:, k, :]
        ball_h = np.stack([b_sh[0], b_sh[1], b_sh[2], b_sp[t - 1]], axis=0)
        per_t[t] = (wt_h, wg_h, np.ascontiguousarray(ball_h))
    return per_t


def _prepare(x, b_seq, W_sh, b_sh, W_sp, b_sp, w_gates, gamma, beta):
    x = np.ascontiguousarray(np.asarray(x, dtype=np.float32))
    b = np.asarray(b_seq).astype(np.int64).ravel()
    W_sh = np.asarray(W_sh, dtype=np.float32)
    b_sh = np.asarray(b_sh, dtype=np.float32)
    W_sp = np.asarray(W_sp, dtype=np.float32)
    b_sp = np.asarray(b_sp, dtype=np.float32)
    w_gates = np.asarray(w_gates, dtype=np.float32)
    gamma = np.asarray(gamma, dtype=np.float32)
    beta = np.asarray(beta, dtype=np.float32)
    assert x.shape == (N, D) and b.shape == (N,)

    trivial = bool(np.all(gamma == 1.0) and np.all(beta == 0.0))
    cores, leftover = _pack_tokens(b)
    per_t = _behavior_tensors(W_sh, b_sh, W_sp, b_sp, w_gates)

    in_maps = []
    for idx, msk, t in cores:
        wt_h, wg_h, ball_h = per_t[t]
        xc = np.ascontiguousarray(x[idx])                      # [M, D]
        # [p, k*M + m] = x[m, 128k + p]
        xt_h = np.ascontiguousarray(
            xc.T.reshape(KT, 128, M).transpose(1, 0, 2).reshape(128, KT * M)
        )
        m = {
            "xt": xt_h,
            "xtb": xt_h.astype(ml_dtypes.bfloat16),
            "xtok": xc,
            "wt": wt_h,
            "wg": wg_h,
            "ball": ball_h,
            "mask": np.ascontiguousarray(msk.reshape(IT, 128).T),
        }
        if not trivial:
            m["gam"] = np.ascontiguousarray(np.broadcast_to(gamma, (128, D)))
            m["bet"] = np.ascontiguousarray(np.broadcast_to(beta, (128, D)))
        in_maps.append(m)
    return trivial, cores, leftover, in_maps, x, beta


def kernel_with_results(trace: bool = False, **inputs):
    trivial, cores, leftover, in_maps, x, beta = _prepare(**inputs)
    nc = _get_program(trivial)
    res = run_bass_kernel_spmd(
        nc, in_maps, list(range(NCORES)), trace=trace
    )
    out = np.empty((N, D), np.float32)
    for c, (idx, _msk, _t) in enumerate(cores):
        out[idx] = res.results[c]["out"]
    if len(leftover):
        out[leftover] = x[leftover] + beta[None, :]
    return out, res


def kernel(**inputs) -> np.ndarray:
    out, _ = kernel_with_results(trace=False, **inputs)
    return out


# revision 13
# speedup vs baseline: 1.3533x; 1.1243x over previous
"""BehaviorMoE Trainium2 kernel (8 NeuronCores, SPMD data-parallel over sorted tokens).

Contract: kernel(**inputs) takes FULL inputs as returned by setup_inputs() and
returns the FULL [8192, 1024] float32 output.

Strategy (v4):
  - Host: sort tokens by behavior id. Each behavior t in 1..4 owns two cores;
    each core gets M=896 tokens (7x128 tiles) of that single behavior, padded
    with masked b==0 filler. Leftover b==0 tokens (out = x + beta) never touch
    the device and are assembled on host.
  - Device (identical SPMD program, per-core data):
      Expert matmuls in bf16 (x and W; ~0.2% rel err); gate logits in fp32r
      off a separate f32 xT (softmax amplifies logit rounding, so bf16 is
      not usable there). All weights resident in SBUF. All input DMAs are
      issued from Sync in arrival-priority order, k-chunked so the opening
      e0/c0 wave streams as the data lands; late inputs (xtok, e2/e3
      weights) are issued behind an SBUF->SBUF dependency DMA on the gates
      so they don't steal HBM bandwidth from the critical path.
      Combine chain per (tile, half): e0 gate-scale (tensor_scalar -> PSUM
      banks recycle as soon as gates land), e1 STT, gate-combined bias
      (exps^T @ b_all as a bf16 PE matmul; PSUM -> SBUF via Scalar copy;
      joined by a GpSimd STT scaled with rm = mask/expsum), e2/e3 STTs
      (ping-pong bf16 SBUF accumulators; in-place DVE ops fault).
      e2+e3 run fused per tile so each tile's LayerNorm (bn_stats halves),
      normalize (ACT halves), residual (GpSimd) and output DMA hide under
      the next tile's matmul stream; the final tile splits its residual
      across GpSimd+Vector to shorten the exposed tail.
  - Host: scatter per-core outputs back to original token order.
"""

import os
import sys

import numpy as np
import ml_dtypes

for _p in ("/opt/trn_rl_repo", "/root/.axon_site/_ro/trn_rl_repo"):
    if os.path.isdir(_p) and _p not in sys.path:
        sys.path.append(_p)

from contextlib import ExitStack

from concourse import bacc, bass, masks, mybir, tile
from concourse.bass_utils import run_bass_kernel_spmd

F32 = mybir.dt.float32
F32R = mybir.dt.float32r
BF16 = mybir.dt.bfloat16
AX = mybir.AxisListType
ALU = mybir.AluOpType
ACTF = mybir.ActivationFunctionType

D = 1024            # model dim
N = 8192            # tokens
NB = 4              # behaviors
NESH = 3            # shared experts
NE = 4              # experts per behavior (3 shared + 1 specific)
EPS = 1e-5
NCORES = 8
M = 896             # tokens per core (7 tiles of 128)
KT = D // 128       # k tiles (contraction)
IT = M // 128       # token tiles per core
FH = 512            # feature half-tile (psum bank width in f32)
C1 = M - FH         # second logit token chunk (384)


def _build_program(trivial_affine: bool) -> bass.Bass:
    nc = bacc.Bacc()

    xt_d = nc.declare_dram_parameter("xt", [128, KT * M], F32R, isOutput=False)
    xtb_d = nc.declare_dram_parameter("xtb", [128, KT * M], BF16, isOutput=False)
    xtok_d = nc.declare_dram_parameter("xtok", [M, D], BF16, isOutput=False)
    wt_d = nc.declare_dram_parameter("wt", [NE, 2, 128, KT * FH], BF16, isOutput=False)
    wg_d = nc.declare_dram_parameter("wg", [128, KT * NE], F32R, isOutput=False)
    ball_d = nc.declare_dram_parameter("ball", [NE, D], BF16, isOutput=False)
    mask_d = nc.declare_dram_parameter("mask", [128, IT], F32, isOutput=False)
    if not trivial_affine:
        gam_d = nc.declare_dram_parameter("gam", [128, D], F32, isOutput=False)
        bet_d = nc.declare_dram_parameter("bet", [128, D], F32, isOutput=False)
    out_d = nc.declare_dram_parameter("out", [M, D], F32, isOutput=True)

    with tile.TileContext(nc) as tc, ExitStack() as ctx:
        const = ctx.enter_context(tc.tile_pool(name="const", bufs=1))
        wpool = ctx.enter_context(tc.tile_pool(name="w", bufs=2 * NE))
        selp = ctx.enter_context(tc.tile_pool(name="sel", bufs=2 * IT))
        xtokp = ctx.enter_context(tc.tile_pool(name="xtok", bufs=IT))
        outp = ctx.enter_context(tc.tile_pool(name="outp", bufs=3))
        scrp = ctx.enter_context(tc.tile_pool(name="scr", bufs=3))
        biasp = ctx.enter_context(tc.tile_pool(name="bias", bufs=3))
        gatep = ctx.enter_context(tc.tile_pool(name="gate", bufs=IT))
        gtsp = ctx.enter_context(tc.tile_pool(name="gts", bufs=2 * IT))
        smallp = ctx.enter_context(tc.tile_pool(name="small", bufs=16))
        zpool = ctx.enter_context(tc.tile_pool(name="z", bufs=6, space="PSUM"))
        pspool = ctx.enter_context(tc.tile_pool(name="ps", bufs=2, space="PSUM"))

        # ---- DMA issues, all on Sync in arrival-priority order ----
        wg_sb = const.tile([128, KT * NE], F32R, tag="wg")
        nc.sync.dma_start(wg_sb[:], wg_d[:])
        xtb = const.tile([128, KT * M], BF16, tag="xtb")
        w_sb = {}
        for e in range(NE):
            for c in (0, 1):
                w_sb[(e, c)] = wpool.tile(
                    [128, KT * FH], BF16, tag="w", name=f"w{e}{c}"
                )
        # opening wave data, k-chunked and interleaved
        for j in range(4):
            xs = slice(j * 2 * M, (j + 1) * 2 * M)
            ws = slice(j * 2 * FH, (j + 1) * 2 * FH)
            nc.sync.dma_start(xtb[:, xs], xtb_d[:, xs])
            nc.sync.dma_start(w_sb[(0, 0)][:, ws], wt_d[0, 0, :, ws])
        xT = const.tile([128, KT * M], F32R, tag="xt")
        half = KT * M // 2
        nc.sync.dma_start(xT[:, 0:half], xt_d[:, 0:half])
        nc.sync.dma_start(xT[:, half:], xt_d[:, half:])
        mask_sb = const.tile([128, IT], F32, tag="mask")
        nc.sync.dma_start(mask_sb[:], mask_d[:])
        ball_sb = const.tile([NE, D], BF16, tag="ball")
        nc.sync.dma_start(ball_sb[:], ball_d[:])
        nc.sync.dma_start(w_sb[(0, 1)][:], wt_d[0, 1])
        nc.sync.dma_start(w_sb[(1, 0)][:], wt_d[1, 0])
        nc.sync.dma_start(w_sb[(1, 1)][:], wt_d[1, 1])
        if not trivial_affine:
            gam_sb = const.tile([128, D], F32, tag="gam")
            nc.sync.dma_start(gam_sb[:], gam_d[:])
            bet_sb = const.tile([128, D], F32, tag="bet")
            nc.sync.dma_start(bet_sb[:], bet_d[:])

        identity = const.tile([128, 128], F32, tag="ident")
        masks.make_identity(nc, identity[:])
        identB = const.tile([128, 128], BF16, tag="identB")
        nc.vector.tensor_copy(identB[:], identity[:])

        # ---- accumulators (ping-pong; in-place DVE ops fault) ----
        selA = [selp.tile([128, D], BF16, tag="sel", name=f"selA{i}")
                for i in range(IT)]
        selB = [selp.tile([128, D], BF16, tag="sel", name=f"selB{i}")
                for i in range(IT)]

        def isl(i):
            return slice(i * 128, (i + 1) * 128)

        def csl(c):
            return slice(c * FH, (c + 1) * FH)

        def expert_mms(zt, e, c, i):
            for k in range(KT):
                nc.tensor.matmul(
                    zt[:], xtb[:, k * M + i * 128:k * M + (i + 1) * 128],
                    w_sb[(e, c)][:, k * FH:(k + 1) * FH],
                    start=(k == 0), stop=(k == KT - 1),
                )

        def expert_group(e, c, i):
            zt = zpool.tile([128, FH], F32, tag="z")
            expert_mms(zt, e, c, i)
            return zt

        # ---- e0/c0 wave for tiles 0..5, k-outer (paced by the k-chunk DMAs) ----
        zt_e0c0 = [zpool.tile([128, FH], F32, tag="z", name=f"zw{i}")
                   for i in range(6)]
        for k in range(KT):
            for i in range(6):
                nc.tensor.matmul(
                    zt_e0c0[i][:], xtb[:, k * M + isl(i).start:k * M + isl(i).stop],
                    w_sb[(0, 0)][:, k * FH:(k + 1) * FH],
                    start=(k == 0), stop=(k == KT - 1),
                )

        # ---- gate logits (fp32r), transpose, masked softmax ----
        glc0 = pspool.tile([128, FH], F32, tag="ps", name="glc0")
        glc1 = pspool.tile([128, FH], F32, tag="ps", name="glc1")
        for k in range(KT):
            nc.tensor.matmul(
                glc0[0:NE, :], wg_sb[:, k * NE:(k + 1) * NE],
                xT[:, k * M:k * M + FH],
                start=(k == 0), stop=(k == KT - 1),
            )
            nc.tensor.matmul(
                glc1[0:NE, 0:C1], wg_sb[:, k * NE:(k + 1) * NE],
                xT[:, k * M + FH:(k + 1) * M],
                start=(k == 0), stop=(k == KT - 1),
            )
        glT_sb = const.tile([NE, M], F32R, tag="glT")
        nc.vector.tensor_copy(glT_sb[:, 0:FH], glc0[0:NE, :])
        nc.vector.tensor_copy(glT_sb[:, FH:M], glc1[0:NE, 0:C1])
        identR = const.tile([NE, NE], F32R, tag="identR")
        nc.vector.tensor_copy(identR[:], identity[0:NE, 0:NE])

        gates_t = []
        exps_t = []
        rm_t = []
        for i in range(IT):
            glp = pspool.tile([128, FH], F32, tag="ps", name=f"glp{i}")
            nc.tensor.matmul(
                glp[:, 0:NE], glT_sb[:, isl(i)], identR[:],
                start=True, stop=True,
            )
            negmax = smallp.tile([128, 1], F32, tag="s1")
            nc.vector.tensor_reduce(
                negmax[:], glp[:, 0:NE], axis=AX.X, op=ALU.max, negate=True
            )
            exps = smallp.tile([128, NE], F32, tag="s4")
            expsum = smallp.tile([128, 1], F32, tag="s1")
            nc.scalar.activation(
                exps[:], glp[:, 0:NE], ACTF.Exp,
                bias=negmax[:], scale=1.0, accum_out=expsum[:],
            )
            rinv = smallp.tile([128, 1], F32, tag="s1")
            nc.vector.reciprocal(rinv[:], expsum[:])
            rm = smallp.tile([128, 1], F32, tag="rm", name=f"rm{i}")
            nc.vector.tensor_mul(rm[:], rinv[:], mask_sb[:, i:i + 1])
            rm_t.append(rm)
            gates = gatep.tile([128, NE], F32, tag="g")
            nc.vector.tensor_scalar_mul(gates[:], exps[:], rm[:])
            gates_t.append(gates)
            exps_t.append(exps)

        # e0/c0 combines for the wave tiles: banks recycle as gates land
        for i in range(6):
            nc.vector.tensor_scalar_mul(
                selB[i][:, csl(0)], zt_e0c0[i][:], gates_t[i][:, 0:1]
            )
        zt6 = expert_group(0, 0, 6)
        nc.vector.tensor_scalar_mul(
            selB[6][:, csl(0)], zt6[:], gates_t[6][:, 0:1]
        )

        # exps^T via PE (bf16) for the bias combine
        expsT_t = []
        for i in range(IT):
            expsB = gtsp.tile([128, NE], BF16, tag="eB", name=f"eB{i}")
            nc.vector.tensor_copy(expsB[:], exps_t[i][:])
            gtp = pspool.tile([128, FH], F32, tag="ps", name=f"gtp{i}")
            nc.tensor.matmul(
                gtp[0:NE, 0:128], expsB[:], identB[:], start=True, stop=True
            )
            expsT = gtsp.tile([NE, 128], BF16, tag="eT", name=f"eT{i}")
            nc.vector.tensor_copy(expsT[:], gtp[0:NE, 0:128])
            expsT_t.append(expsT)

        # late inputs: gate their issue on gates_t[0] so they don't steal
        # HBM bandwidth from the opening critical path
        depgate = const.tile([128, NE], F32, tag="depg")
        nc.sync.dma_start(depgate[:], gates_t[0][:])
        xtok_t = []
        for i in range(IT):
            xi = xtokp.tile([128, D], BF16, tag="xtok", name=f"xi{i}")
            nc.sync.dma_start(xi[:], xtok_d[isl(i), :])
            xtok_t.append(xi)
        for e in (2, 3):
            for c in (0, 1):
                nc.sync.dma_start(w_sb[(e, c)][:], wt_d[e, c])

        # ---- e0/c1 ----
        for i in range(IT):
            zt = expert_group(0, 1, i)
            nc.vector.tensor_scalar_mul(
                selB[i][:, csl(1)], zt[:], gates_t[i][:, 0:1]
            )

        # ---- e1 (+ bias combine join per (c, tile)) ----
        for c in (0, 1):
            for i in range(IT):
                zt = expert_group(1, c, i)
                nc.vector.scalar_tensor_tensor(
                    selA[i][:, csl(c)], zt[:], gates_t[i][:, 1:2],
                    selB[i][:, csl(c)], op0=ALU.mult, op1=ALU.add,
                )
                bp = pspool.tile([128, FH], F32, tag="ps", name=f"bp{i}{c}")
                nc.tensor.matmul(
                    bp[:], expsT_t[i][:], ball_sb[:, csl(c)],
                    start=True, stop=True,
                )
                bias_sb = biasp.tile([128, FH], F32, tag="bias")
                nc.scalar.copy(bias_sb[:], bp[:])
                nc.vector.scalar_tensor_tensor(
                    selB[i][:, csl(c)], bias_sb[:], rm_t[i][:],
                    selA[i][:, csl(c)], op0=ALU.mult, op1=ALU.add,
                )

        # ---- fused e2+e3 passes with per-tile LN tail ----
        for i in range(IT):
            bn6 = smallp.tile([128, 2 * 6], F32, tag="bn6")
            for c in (0, 1):
                zt2 = expert_group(2, c, i)
                nc.vector.scalar_tensor_tensor(
                    selA[i][:, csl(c)], zt2[:], gates_t[i][:, 2:3],
                    selB[i][:, csl(c)], op0=ALU.mult, op1=ALU.add,
                )
                zt3 = expert_group(3, c, i)
                nc.vector.scalar_tensor_tensor(
                    selB[i][:, csl(c)], zt3[:], gates_t[i][:, 3:4],
                    selA[i][:, csl(c)], op0=ALU.mult, op1=ALU.add,
                )
                nc.vector.bn_stats(bn6[:, 6 * c:6 * c + 6], selB[i][:, csl(c)])
            selF = selB[i]
            mv = smallp.tile([128, 2], F32, tag="mv")
            nc.vector.bn_aggr(mv[:], bn6[:])
            avi = smallp.tile([128, 1], F32, tag="s1")
            nc.vector.tensor_scalar_add(avi[:], mv[:, 1:2], EPS)
            sdi = smallp.tile([128, 1], F32, tag="s1")
            nc.scalar.sqrt(sdi[:], avi[:])
            ri = smallp.tile([128, 1], F32, tag="s1")
            nc.vector.reciprocal(ri[:], sdi[:])
            mbt = smallp.tile([128, 1], F32, tag="s1")
            nc.vector.tensor_mul(mbt[:], mv[:, 0:1], ri[:])
            mbi = smallp.tile([128, 1], F32, tag="s1")
            nc.vector.tensor_scalar_mul(mbi[:], mbt[:], -1.0)
            # ln = sel*rstd + mb on ACT (halves), residual on GpSimd
            # (last tile: split across GpSimd+Vector to shorten the tail)
            xi = xtok_t[i]
            outt = outp.tile([128, D], F32, tag="out")
            for c in (0, 1):
                lnb = scrp.tile([128, FH], F32, tag="scr")
                nc.scalar.activation(
                    lnb[:], selF[:, csl(c)], ACTF.Identity,
                    bias=mbi[:], scale=ri[:],
                )
                if not trivial_affine:
                    lng = scrp.tile([128, FH], F32, tag="scr")
                    nc.vector.tensor_mul(lng[:], lnb[:], gam_sb[:, csl(c)])
                    lnb2 = scrp.tile([128, FH], F32, tag="scr")
                    nc.vector.tensor_add(lnb2[:], lng[:], bet_sb[:, csl(c)])
                    lnb = lnb2
                eng = nc.vector if (c == 1 and i == IT - 1) else nc.gpsimd
                eng.tensor_add(outt[:, csl(c)], lnb[:], xi[:, csl(c)])
                nc.sync.dma_start(out_d[isl(i), csl(c)], outt[:, csl(c)])

    nc.finalize()
    return nc


_PROGRAM_CACHE: dict = {}


def _get_program(trivial_affine: bool) -> bass.Bass:
    key = trivial_affine
    if key not in _PROGRAM_CACHE:
        _PROGRAM_CACHE[key] = _build_program(trivial_affine)
    return _PROGRAM_CACHE[key]


def _pack_tokens(b: np.ndarray):
    """Two cores per behavior t in 1..4, M=896 tokens each, padded with masked
    b==0 filler. Returns (per-core (idx, mask, t) list, leftover b==0 idx)."""
    idx0 = np.flatnonzero(b == 0)
    p0 = 0
    cores = []
    for t in range(1, NB + 1):
        idxs = np.flatnonzero(b == t)
        if len(idxs) > 2 * M:
            raise RuntimeError(
                f"behavior {t} has {len(idxs)} tokens > capacity {2 * M}"
            )
        for s in (0, M):
            part = idxs[s:s + M]
            need = M - len(part)
            fill = idx0[p0:p0 + need]
            p0 += need
            if len(fill) != need:
                raise RuntimeError("not enough b==0 filler tokens for packing")
            idx = np.concatenate([part.astype(np.int64), fill.astype(np.int64)])
            msk = np.zeros((M,), np.float32)
            msk[:len(part)] = 1.0
            cores.append((idx, msk, t))
    return cores, idx0[p0:]


def _behavior_tensors(W_sh, b_sh, W_sp, b_sp, w_gates):
    per_t = {}
    W_sh_flat = W_sh.reshape(NESH * D, D)
    for t in range(1, NB + 1):
        Wall = np.concatenate([W_sh_flat, W_sp[t - 1:t].reshape(D, D)], axis=0)
        wT = np.ascontiguousarray(Wall.T)                      # [D, NE*D]
        # [e, c, p, k*FH + f] = wT[128k + p, e*D + c*FH + f]
        wt_h = np.ascontiguousarray(
            wT.reshape(KT, 128, NE, 2, FH).transpose(2, 3, 1, 0, 4)
            .reshape(NE, 2, 128, KT * FH).astype(ml_dtypes.bfloat16)
        )
        # [p, k*NE + e] = w_gates[t-1][128k + p, e]
        wg_h = np.ascontiguousarray(
            w_gates[t - 1].reshape(KT, 128, NE).transpose(1, 0, 2)
            .reshape(128, KT * NE)
        )
        ball_h = np.stack([b_sh[0], b_sh[1], b_sh[2], b_sp[t - 1]], axis=0)
        per_t[t] = (wt_h, wg_h,
                    np.ascontiguousarray(ball_h).astype(ml_dtypes.bfloat16))
    return per_t


def _prepare(x, b_seq, W_sh, b_sh, W_sp, b_sp, w_gates, gamma, beta):
    x = np.ascontiguousarray(np.asarray(x, dtype=np.float32))
    b = np.asarray(b_seq).astype(np.int64).ravel()
    W_sh = np.asarray(W_sh, dtype=np.float32)
    b_sh = np.asarray(b_sh, dtype=np.float32)
    W_sp = np.asarray(W_sp, dtype=np.float32)
    b_sp = np.asarray(b_sp, dtype=np.float32)
    w_gates = np.asarray(w_gates, dtype=np.float32)
    gamma = np.asarray(gamma, dtype=np.float32)
    beta = np.asarray(beta, dtype=np.float32)
    assert x.shape == (N, D) and b.shape == (N,)

    trivial = bool(np.all(gamma == 1.0) and np.all(beta == 0.0))
    cores, leftover = _pack_tokens(b)
    per_t = _behavior_tensors(W_sh, b_sh, W_sp, b_sp, w_gates)

    in_maps = []
    for idx, msk, t in cores:
        wt_h, wg_h, ball_h = per_t[t]
        xc = np.ascontiguousarray(x[idx])                      # [M, D]
        # [p, k*M + m] = x[m, 128k + p]
        xt_h = np.ascontiguousarray(
            xc.T.reshape(KT, 128, M).transpose(1, 0, 2).reshape(128, KT * M)
        )
        m = {
            "xt": xt_h,
            "xtb": xt_h.astype(ml_dtypes.bfloat16),
            "xtok": xc.astype(ml_dtypes.bfloat16),
            "wt": wt_h,
            "wg": wg_h,
            "ball": ball_h,
            "mask": np.ascontiguousarray(msk.reshape(IT, 128).T),
        }
        if not trivial:
            m["gam"] = np.ascontiguousarray(np.broadcast_to(gamma, (128, D)))
            m["bet"] = np.ascontiguousarray(np.broadcast_to(beta, (128, D)))
        in_maps.append(m)
    return trivial, cores, leftover, in_maps, x, beta


def kernel_with_results(trace: bool = False, **inputs):
    trivial, cores, leftover, in_maps, x, beta = _prepare(**inputs)
    nc = _get_program(trivial)
    res = run_bass_kernel_spmd(
        nc, in_maps, list(range(NCORES)), trace=trace
    )
    out = np.empty((N, D), np.float32)
    for c, (idx, _msk, _t) in enumerate(cores):
        out[idx] = res.results[c]["out"]
    if len(leftover):
        out[leftover] = x[leftover] + beta[None, :]
    return out, res


def kernel(**inputs) -> np.ndarray:
    out, _ = kernel_with_results(trace=False, **inputs)
    return out
